# revision 1
# baseline (speedup 1.0000x reference)
"""Trainium2 Bass kernel for nn_CAM (channel attention module).

Reference computation (per batch element n):
    v = x[n].reshape(C, H*W)                      # [512, 4096]
    energy = v @ v.T                              # [512, 512]
    attn = softmax(energy, axis=-1)
    out = attn @ v                                # [512, 4096]
    result = para_mu * out + x[n]

Sharding: data-parallel over batch N=8, one batch element per NeuronCore
(8 cores). Everything is core-local — no collectives.

Kernel strategy (per core):
  1. Chunked DMA of v (natural layout, C on partitions) into SBUF as 4
     [128, 4096] tiles, column-slab interleaved so compute starts early.
  2. Build vT column slabs via TensorE transposes in float32r (no fp32 DMA
     transpose on trn2); the 4 transposes of one slab land in a single PSUM
     bank and move to SBUF with one DVE copy, software-pipelined one slab
     ahead of the matmuls. All matmuls run float32r (FP22, 1 cycle/row at
     moving dim >= 256).
  3. Energy = vT.T @ vT, m-outer and symmetric: row tile m computes only
     column blocks j >= [0,1,2,2][m]; the 5 missing lower blocks are
     transposed twins. E[0] completes right after the transpose stream, so
     softmax of row 0 overlaps the remaining energy matmuls.
  4. Row softmax: reduce_max (negated) -> Exp activation with per-partition
     bias + accumulated row sum -> reciprocal. exp rows stay unnormalized;
     the epilogue scale carries para_mu/rowsum.
  5. Transpose exp -> expT (TensorE, one row tile ahead of its matmuls);
     out_unnorm = expT.T @ v with accumulators rotating over 6 PSUM banks.
  6. Epilogue split across engines: ACT scales out of PSUM, DVE adds the
     residual x at SBUF 2x rate; results ship as 1 MB / 512 KB DMAs.
"""

import sys

if "/opt/trn_rl_repo" not in sys.path:
    sys.path.insert(0, "/opt/trn_rl_repo")

from contextlib import ExitStack

import numpy as np

import concourse.bass as bass
import concourse.mybir as mybir
import concourse.tile as tile
from concourse import bacc
from concourse.bass_utils import run_bass_kernel_spmd
from concourse.masks import make_identity

N, C, H, W = 8, 512, 64, 64
HW = H * W            # 4096
P = 128               # partitions
MT = C // P           # 4 row tiles of the channel dim
KT = HW // P          # 32 contraction tiles for the energy matmul
NCH = 512             # free-dim chunk for the output matmul (one PSUM bank)
NCHUNKS = HW // NCH   # 8
F32 = mybir.dt.float32
F32R = mybir.dt.float32r


def _body(ctx: ExitStack, tc: "tile.TileContext", out: bass.AP, x: bass.AP, pm: bass.AP,
          reps: int = 1):
    nc = tc.nc
    consts = ctx.enter_context(tc.tile_pool(name="consts", bufs=1))
    v_pool = ctx.enter_context(tc.tile_pool(name="v", bufs=1))
    vt_pool = ctx.enter_context(tc.tile_pool(name="vt", bufs=1))
    exp_pool = ctx.enter_context(tc.tile_pool(name="exp", bufs=1))
    expt_pool = ctx.enter_context(tc.tile_pool(name="expt", bufs=1))
    stat_pool = ctx.enter_context(tc.tile_pool(name="stats", bufs=1))
    out_pool = ctx.enter_context(tc.tile_pool(name="ob", bufs=2))
    e_psum = ctx.enter_context(tc.tile_pool(name="e_ps", bufs=1, space="PSUM"))
    t_psum = ctx.enter_context(tc.tile_pool(name="t_ps", bufs=2, space="PSUM"))
    o_psum = ctx.enter_context(tc.tile_pool(name="o_ps", bufs=2, space="PSUM"))

    identity = consts.tile([P, P], F32)
    nc.vector.memset(identity, 0.0)
    make_identity(nc, identity, nomemset=True)
    # f32r twin for transpose-mode matmuls (1.5 cycles/row vs 2 for fp32);
    # the DVE copy is the sanctioned "rounds to f32r" producer.
    identity_r = consts.tile([P, P], F32R)
    nc.vector.tensor_copy(out=identity_r, in_=identity)

    # emitted after make_identity: the gpsimd queue is serial, and this DMA
    # ahead of affine_select would delay the first transposes by ~1 us
    pm_tile = consts.tile([P, 1], F32)
    nc.gpsimd.dma_start(out=pm_tile, in_=pm.to_broadcast((P, 1)))

    if reps > 1:
        # Benchmark mode: execute the body `reps` times in one NEFF via a
        # hardware loop so per-rep time is measurable over dispatch overhead.
        with tc.For_i(0, reps, 1, hint_engines=(mybir.EngineType.PE,
                                                mybir.EngineType.DVE,
                                                mybir.EngineType.Activation)):
            _phases(tc, out, x, pm_tile, identity, identity_r,
                    consts, v_pool, vt_pool, exp_pool, expt_pool, stat_pool,
                    out_pool, e_psum, t_psum, o_psum)
    else:
        _phases(tc, out, x, pm_tile, identity, identity_r,
                consts, v_pool, vt_pool, exp_pool, expt_pool, stat_pool,
                out_pool, e_psum, t_psum, o_psum)


def _phases(tc, out, x, pm_tile, identity, identity_r,
            consts, v_pool, vt_pool, exp_pool, expt_pool, stat_pool,
            out_pool, e_psum, t_psum, o_psum):
    nc = tc.nc
    # Load v in natural layout: 4 tiles of [128, 4096]. Declared float32r so
    # the BIR verifier accepts it as a matmul-2 rhs; the DMA is a pure bitcast
    # (bits unchanged), and fp32-exact readers bitcast back to F32.
    # Loads are chunked into column slabs, interleaved across the 4 row tiles,
    # so the phase-1 k-loop (which consumes one column slab of all 4 tiles per
    # step) can start after ~1/8 of the data has landed.
    LCH = HW // 8  # 512-column load chunks
    V = [v_pool.tile([P, HW], F32R, name=f"v{m}", tag=f"v{m}") for m in range(MT)]
    for c in range(8):
        for m in range(MT):
            nc.sync.dma_start(
                out=V[m][:, c * LCH:(c + 1) * LCH],
                in_=x[m * P:(m + 1) * P, c * LCH:(c + 1) * LCH].bitcast(F32R),
            )

    # Phase 1: per k-block, transpose the [512, 128] column slab of v into
    # vT[k] [128, 512], then accumulate energy[m] += vT[k][:, m].T @ vT[k].
    # The 4 transposes of one k-block land in a single [128, 4, 128] PSUM bank
    # tile and move to SBUF with ONE DVE copy (keeps DVE off the critical
    # path).
    # Energy is symmetric: row tile m only computes column blocks j >= SYM_LO[m]
    # (m=3 widened to 2 blocks so its moving dim stays >= 256 — f32r drops to
    # 1/4 rate below that). Missing lower blocks are transposes of computed
    # upper blocks.
    SYM_LO = [0, 1, 2, 2]
    E = [e_psum.tile([P, C], F32, name=f"e{m}", tag=f"e{m}") for m in range(MT)]
    # vt slabs stay resident (64 KB/partition) and energy runs m-outer:
    # E[0] finishes as soon as the transpose stream ends, so the whole
    # softmax/expT chain for row tile 0 hides behind the energy matmuls of
    # row tiles 1-3. Transposes are software-pipelined one k ahead of the
    # m=0 matmuls to cover the vt PSUM->SBUF copy latency.
    vts = [vt_pool.tile([P, C], F32R, name=f"vt{k}", tag=f"vt{k}") for k in range(KT)]
    for k in range(KT + 1):
        if k < KT:
            tp = t_psum.tile([P, MT, P], F32R, tag="tp")
            for m in range(MT):
                nc.tensor.transpose(
                    tp[:, m, :], V[m][:, k * P:(k + 1) * P], identity_r
                )
            nc.vector.tensor_copy(
                out=vts[k].rearrange("p (m q) -> p m q", m=MT), in_=tp
            )
        if k >= 1:
            kk = k - 1
            nc.tensor.matmul(
                E[0],
                lhsT=vts[kk][:, 0:P],
                rhs=vts[kk],
                start=(kk == 0),
                stop=(kk == KT - 1),
            )
    for m in range(1, MT):
        lo = SYM_LO[m] * P
        for k in range(KT):
            nc.tensor.matmul(
                E[m][:, lo:],
                lhsT=vts[k][:, m * P:(m + 1) * P],
                rhs=vts[k][:, lo:],
                start=(k == 0),
                stop=(k == KT - 1),
            )

    # Per row tile mi: reconstruct full energy row in SBUF (copy computed part
    # + transposed twins of missing lower blocks), row softmax stats, expT
    # transposes, then the output matmuls + fused epilogue. Emitted per-mi so
    # row 0's chain starts while rows 1-3 are still accumulating energy.
    RECON = {0: [], 1: [(1, 0)], 2: [(2, 0), (2, 1)], 3: [(3, 0), (3, 1)]}
    # PSUM->SBUF copies of every computed energy part happen up front: the
    # phase-4 accumulator rotation below reuses the energy banks, so they must
    # all be drained before the first output matmuls run.
    E_sb = []
    for m in range(MT):
        esb = exp_pool.tile([P, C], F32, name=f"esb{m}", tag=f"esb{m}")
        nc.vector.tensor_copy(out=esb[:, SYM_LO[m] * P:], in_=E[m][:, SYM_LO[m] * P:])
        E_sb.append(esb)
    # Softmax stats for every row tile, hoisted ahead of phase 4 so the DVE's
    # in-order queue never makes a later row's stats wait behind an earlier
    # row's epilogue. exp rows stay unnormalized (epilogue carries pm/sum).
    EXP = []
    SCALE = []
    for mi in range(MT):
        for ti, tj in RECON[mi]:
            tp = t_psum.tile([P, MT, P], F32, tag="tp")
            nc.tensor.transpose(tp[:, 0, :], E_sb[tj][:, ti * P:(ti + 1) * P], identity)
            nc.vector.tensor_copy(out=E_sb[ti][:, tj * P:(tj + 1) * P], in_=tp[:, 0, :])
        neg_max = stat_pool.tile([P, 1], F32, tag=f"negm{mi}")
        nc.vector.tensor_reduce(
            out=neg_max,
            in_=E_sb[mi],
            op=mybir.AluOpType.max,
            axis=mybir.AxisListType.X,
            negate=True,
        )
        exp_t = exp_pool.tile([P, C], F32, name=f"exp{mi}", tag=f"exp{mi}")
        s_t = stat_pool.tile([P, 1], F32, tag=f"s{mi}")
        nc.scalar.activation(
            out=exp_t,
            in_=E_sb[mi],
            func=mybir.ActivationFunctionType.Exp,
            bias=neg_max,
            scale=1.0,
            accum_out=s_t,
        )
        rs = stat_pool.tile([P, 1], F32, tag=f"rs{mi}")
        nc.vector.reciprocal(rs, s_t)
        sc = stat_pool.tile([P, 1], F32, tag=f"sc{mi}")
        nc.vector.tensor_mul(sc, rs, pm_tile)
        EXP.append(exp_t)
        SCALE.append(sc)

    # Phase 4: out rows = expT.T @ v. Each row tile's expT transposes are
    # emitted just before its matmuls: row 0's output stream starts while
    # rows 1-3 exp chains are still finishing on ACT/DVE. Accumulators rotate
    # over 6 PSUM banks (2 o-banks + the 4 energy banks, dead after the exp
    # pass). Epilogue is split across engines: ACT does the PSUM read +
    # pm/sum scale, DVE adds the residual x at SBUF 2x rate. Results stage
    # into a [128, 4096] tile, shipped as 1 MB DMAs (the closing half in two
    # 512 KB pieces to shorten the kernel tail).
    EXPT = expt_pool.tile([P, MT, C], F32R, name="expt", tag="expt")

    def emit_expt_block(mi):
        tp = t_psum.tile([P, MT, P], F32, tag="tp", name=f"tpx{mi}")
        for mj in range(MT):
            nc.tensor.transpose(tp[:, mj, :], EXP[mi][:, mj * P:(mj + 1) * P], identity)
        nc.vector.tensor_copy(out=EXPT[:, :, mi * P:(mi + 1) * P], in_=tp)

    emit_expt_block(0)
    for mi in range(MT):
        if mi + 1 < MT:
            emit_expt_block(mi + 1)  # one row ahead: copy overlaps mi's matmuls
        ob = out_pool.tile([P, HW], F32, tag="ob")
        for cidx in range(NCHUNKS):
            slot = (mi * NCHUNKS + cidx) % 6
            if slot < 4:
                o_ps = e_psum.tile([P, NCH], F32, name=f"ops{slot}", tag=f"e{slot}")
            else:
                o_ps = o_psum.tile([P, NCH], F32, name=f"ops{slot}", tag="ops")
            for mj in range(MT):
                nc.tensor.matmul(
                    o_ps,
                    lhsT=EXPT[:, mj, mi * P:(mi + 1) * P],
                    rhs=V[mj][:, cidx * NCH:(cidx + 1) * NCH],
                    start=(mj == 0),
                    stop=(mj == MT - 1),
                )
            obc = ob[:, cidx * NCH:(cidx + 1) * NCH]
            nc.scalar.mul(obc, o_ps, SCALE[mi])
            nc.vector.tensor_add(
                obc, obc, V[mi].bitcast(F32)[:, cidx * NCH:(cidx + 1) * NCH]
            )
            if cidx == NCHUNKS // 2 - 1:
                nc.sync.dma_start(
                    out=out[mi * P:(mi + 1) * P, :HW // 2], in_=ob[:, :HW // 2]
                )
            elif cidx == NCHUNKS - 3:
                nc.sync.dma_start(
                    out=out[mi * P:(mi + 1) * P, HW // 2:HW * 3 // 4],
                    in_=ob[:, HW // 2:HW * 3 // 4],
                )
        nc.sync.dma_start(
            out=out[mi * P:(mi + 1) * P, HW * 3 // 4:], in_=ob[:, HW * 3 // 4:]
        )


def build_nc(reps: int = 1) -> bass.Bass:
    # bacc.Bacc (not raw bass.Bass): its compile() pass legalizes multi-sem
    # waits into explicit event-semaphore instructions (walrus allows only one
    # sync wait per TPB instruction).
    nc = bacc.Bacc("TRN2", debug=False)
    x = nc.dram_tensor("x", [C, HW], F32, kind="ExternalInput").ap()
    pm = nc.dram_tensor("para_mu", [1], F32, kind="ExternalInput").ap()
    out = nc.dram_tensor("out", [C, HW], F32, kind="ExternalOutput").ap()
    with tile.TileContext(nc) as tc, ExitStack() as ctx:
        _body(ctx, tc, out, x, pm, reps=reps)
    nc.compile()
    return nc


_nc_cache = None


def run(x: np.ndarray, para_mu: np.ndarray, **spmd_kwargs):
    """Run on 8 NeuronCores; returns (output [8,512,64,64], BassKernelResults)."""
    global _nc_cache
    x = np.ascontiguousarray(np.asarray(x, dtype=np.float32))
    pm = np.ascontiguousarray(np.asarray(para_mu, dtype=np.float32).reshape(1))
    assert x.shape == (N, C, H, W), x.shape
    if _nc_cache is None:
        _nc_cache = build_nc()
    in_maps = [
        {"x": x[n].reshape(C, HW), "para_mu": pm} for n in range(N)
    ]
    res = run_bass_kernel_spmd(_nc_cache, in_maps, core_ids=list(range(N)), **spmd_kwargs)
    out = np.stack(
        [np.asarray(res.results[n]["out"]).reshape(C, H, W) for n in range(N)]
    )
    return out, res


def kernel(x: np.ndarray, para_mu: np.ndarray) -> np.ndarray:
    out, _ = run(x, para_mu)
    return out



# revision 2
# speedup vs baseline: 1.1102x; 1.1102x over previous
"""Trainium2 Bass kernel for nn_CAM (channel attention module).

Reference computation (per batch element n):
    v = x[n].reshape(C, H*W)                      # [512, 4096]
    energy = v @ v.T                              # [512, 512]
    attn = softmax(energy, axis=-1)
    out = attn @ v                                # [512, 4096]
    result = para_mu * out + x[n]

Sharding: data-parallel over batch N=8, one batch element per NeuronCore
(8 cores). Everything is core-local — no collectives.

v2 kernel strategy (per core) — bf16 pipeline:
  1. Input lands as bf16 via SWDGE cast-DMA (f32 HBM read, bf16 SBUF write):
     8 column-slab DMAs into a single [128, 4, 4096] tile so the transpose
     stream starts after ~1/8 of the data. rel-err budget is 2e-2; bf16
     everywhere costs ~2e-3.
  2. vT column slabs via TensorE transposes in bf16 (1 cycle/row, vs 1.5 for
     f32r); the 4 transposes of one slab land in one PSUM tile and move to
     SBUF with one DVE copy (16-bit 2x rate), pipelined one slab ahead.
  3. Energy = vT.T @ vT, m-outer and symmetric (row tile m computes column
     blocks j >= [0,1,2,2][m]); missing lower blocks are transposed twins.
     E[0] completes right after the transpose stream so row 0's softmax
     overlaps rows 1-3's energy matmuls. Energy rows stay f32 for softmax.
  4. Row softmax: reduce_max (negated) -> Exp activation writing bf16 with
     f32 accumulated row sum -> reciprocal. exp rows stay unnormalized; the
     epilogue scale carries para_mu/rowsum.
  5. Transpose exp -> expT (bf16), out_unnorm = expT.T @ v (bf16 matmuls,
     f32 PSUM) with accumulators rotating over 6 PSUM banks.
  6. Epilogue: ACT scales PSUM -> bf16, DVE adds the bf16 residual at 16-bit
     2x rate; the output ships to HBM as bf16 (half the store traffic; the
     host upcasts to f32).
  7. Benchmark loop is 2x-unrolled with the V tile double-buffered, so rep
     i+1's input cast-DMA overlaps rep i's output matmuls instead of
     serializing behind the last residual read of V.
"""

import sys

if "/opt/trn_rl_repo" not in sys.path:
    sys.path.insert(0, "/opt/trn_rl_repo")

from contextlib import ExitStack

import numpy as np

import concourse.bass as bass
import concourse.mybir as mybir
import concourse.tile as tile
from concourse import bacc
from concourse.bass_utils import run_bass_kernel_spmd
from concourse.masks import make_identity

N, C, H, W = 8, 512, 64, 64
HW = H * W            # 4096
P = 128               # partitions
MT = C // P           # 4 row tiles of the channel dim
KT = HW // P          # 32 contraction tiles for the energy matmul
NCH = 512             # free-dim chunk for the output matmul (one PSUM bank)
NCHUNKS = HW // NCH   # 8
LSLABS = 8            # input load slabs (512 columns each)
F32 = mybir.dt.float32
BF16 = mybir.dt.bfloat16


def _body(ctx: ExitStack, tc: "tile.TileContext", out: bass.AP, x: bass.AP, pm: bass.AP,
          reps: int = 1):
    nc = tc.nc
    consts = ctx.enter_context(tc.tile_pool(name="consts", bufs=1))
    v_pool = ctx.enter_context(tc.tile_pool(name="v", bufs=2))
    vt_pool = ctx.enter_context(tc.tile_pool(name="vt", bufs=1))
    exp_pool = ctx.enter_context(tc.tile_pool(name="exp", bufs=1))
    expt_pool = ctx.enter_context(tc.tile_pool(name="expt", bufs=1))
    stat_pool = ctx.enter_context(tc.tile_pool(name="stats", bufs=1))
    out_pool = ctx.enter_context(tc.tile_pool(name="ob", bufs=2))
    e_psum = ctx.enter_context(tc.tile_pool(name="e_ps", bufs=1, space="PSUM"))
    t_psum = ctx.enter_context(tc.tile_pool(name="t_ps", bufs=2, space="PSUM"))
    o_psum = ctx.enter_context(tc.tile_pool(name="o_ps", bufs=2, space="PSUM"))

    identity = consts.tile([P, P], F32)
    nc.vector.memset(identity, 0.0)
    make_identity(nc, identity, nomemset=True)
    # bf16 twin for transpose-mode matmuls of bf16 data (1 cycle/row).
    identity_bf = consts.tile([P, P], BF16)
    nc.vector.tensor_copy(out=identity_bf, in_=identity)

    # emitted after make_identity: the gpsimd queue is serial, and this DMA
    # ahead of affine_select would delay the first transposes
    pm_tile = consts.tile([P, 1], F32)
    nc.gpsimd.dma_start(out=pm_tile, in_=pm.to_broadcast((P, 1)))

    pools = (consts, v_pool, vt_pool, exp_pool, expt_pool, stat_pool,
             out_pool, e_psum, t_psum, o_psum)
    if reps > 1:
        # Benchmark mode: execute the body `reps` times in one NEFF via a
        # hardware loop so per-rep time is measurable over dispatch overhead.
        # 2x-unrolled so the double-buffered pools rotate: rep i+1's input
        # DMA overlaps rep i's compute.
        assert reps % 2 == 0, reps
        with tc.For_i(0, reps // 2, 1, hint_engines=(mybir.EngineType.PE,
                                                     mybir.EngineType.DVE,
                                                     mybir.EngineType.Activation)):
            _phases(tc, out, x, pm_tile, identity, identity_bf, *pools)
            _phases(tc, out, x, pm_tile, identity, identity_bf, *pools)
    else:
        _phases(tc, out, x, pm_tile, identity, identity_bf, *pools)


def _phases(tc, out, x, pm_tile, identity, identity_bf,
            consts, v_pool, vt_pool, exp_pool, expt_pool, stat_pool,
            out_pool, e_psum, t_psum, o_psum):
    nc = tc.nc
    # Load v as bf16 in natural layout: one [128, 4, 4096] tile ([p, m, w],
    # channel row-tile m on the free axis). The SWDGE cast-DMA reads f32 from
    # HBM and writes bf16; 8 column slabs so the phase-1 k-loop (which
    # consumes one 128-col block of all 4 row tiles per step) starts after
    # ~1/8 of the data has landed.
    LCH = HW // LSLABS  # 512-column load slabs
    V = v_pool.tile([P, MT, HW], BF16, name="v", tag="v")
    xv = x.rearrange("(m p) w -> p m w", p=P)
    for c in range(LSLABS):
        nc.gpsimd.dma_start(
            out=V[:, :, c * LCH:(c + 1) * LCH],
            in_=xv[:, :, c * LCH:(c + 1) * LCH],
        )

    # Phase 1: per k-block, transpose the [512, 128] column slab of v into
    # vts[:, k, :] [128, 512], then accumulate energy[m] += vT[k][:, m].T @
    # vT[k]. The 4 transposes of one k-block land in a single [128, 4, 128]
    # PSUM tile and move to SBUF with ONE DVE copy.
    # Energy is symmetric: row tile m only computes column blocks j >=
    # SYM_LO[m] (m=3 widened to 2 blocks to stay matmul-stream bound rather
    # than weight-load bound). Missing lower blocks are transposes of
    # computed upper blocks.
    SYM_LO = [0, 1, 2, 2]
    E = [e_psum.tile([P, C], F32, name=f"e{m}", tag=f"e{m}") for m in range(MT)]
    # vt slabs stay resident (32 KB/partition bf16) and energy runs m-outer:
    # E[0] finishes as soon as the transpose stream ends, so the whole
    # softmax/expT chain for row tile 0 hides behind the energy matmuls of
    # row tiles 1-3. Transposes are software-pipelined one k ahead of the
    # m=0 matmuls to cover the vt PSUM->SBUF copy latency.
    vts = vt_pool.tile([P, KT, C], BF16, name="vts", tag="vts")
    for k in range(KT + 1):
        if k < KT:
            tp = t_psum.tile([P, MT, P], BF16, tag="tp")
            for m in range(MT):
                nc.tensor.transpose(
                    tp[:, m, :], V[:, m, k * P:(k + 1) * P], identity_bf
                )
            nc.vector.tensor_copy(
                out=vts[:, k, :].rearrange("p (m q) -> p m q", m=MT), in_=tp
            )
        if k >= 1:
            kk = k - 1
            nc.tensor.matmul(
                E[0],
                lhsT=vts[:, kk, 0:P],
                rhs=vts[:, kk, :],
                start=(kk == 0),
                stop=(kk == KT - 1),
            )
    for m in range(1, MT):
        lo = SYM_LO[m] * P
        for k in range(KT):
            nc.tensor.matmul(
                E[m][:, lo:],
                lhsT=vts[:, k, m * P:(m + 1) * P],
                rhs=vts[:, k, lo:],
                start=(k == 0),
                stop=(k == KT - 1),
            )

    # Per row tile mi: reconstruct full energy row in SBUF (copy computed
    # part + transposed twins of missing lower blocks), row softmax stats,
    # then (phase 4) expT transposes + output matmuls + fused epilogue.
    RECON = {0: [], 1: [(1, 0)], 2: [(2, 0), (2, 1)], 3: [(3, 0), (3, 1)]}
    # PSUM->SBUF copies of every computed energy part happen up front: the
    # phase-4 accumulator rotation below reuses the energy banks, so they
    # must all be drained before the first output matmuls run. Energy rows
    # stay f32 (absolute logit errors turn into exp-scale errors).
    E_sb = []
    for m in range(MT):
        esb = exp_pool.tile([P, C], F32, name=f"esb{m}", tag=f"esb{m}")
        nc.vector.tensor_copy(out=esb[:, SYM_LO[m] * P:], in_=E[m][:, SYM_LO[m] * P:])
        E_sb.append(esb)
    # Softmax stats for every row tile, hoisted ahead of phase 4 so the
    # DVE's in-order queue never makes a later row's stats wait behind an
    # earlier row's epilogue. exp rows stay unnormalized (epilogue carries
    # pm/sum) and are written bf16 for the output matmul.
    EXP = []
    SCALE = []
    for mi in range(MT):
        for ti, tj in RECON[mi]:
            tp = t_psum.tile([P, MT, P], F32, tag="tp")
            nc.tensor.transpose(tp[:, 0, :], E_sb[tj][:, ti * P:(ti + 1) * P], identity)
            nc.vector.tensor_copy(out=E_sb[ti][:, tj * P:(tj + 1) * P], in_=tp[:, 0, :])
        neg_max = stat_pool.tile([P, 1], F32, tag=f"negm{mi}")
        nc.vector.tensor_reduce(
            out=neg_max,
            in_=E_sb[mi],
            op=mybir.AluOpType.max,
            axis=mybir.AxisListType.X,
            negate=True,
        )
        exp_t = exp_pool.tile([P, C], BF16, name=f"exp{mi}", tag=f"exp{mi}")
        s_t = stat_pool.tile([P, 1], F32, tag=f"s{mi}")
        nc.scalar.activation(
            out=exp_t,
            in_=E_sb[mi],
            func=mybir.ActivationFunctionType.Exp,
            bias=neg_max,
            scale=1.0,
            accum_out=s_t,
        )
        rs = stat_pool.tile([P, 1], F32, tag=f"rs{mi}")
        nc.vector.reciprocal(rs, s_t)
        sc = stat_pool.tile([P, 1], F32, tag=f"sc{mi}")
        nc.vector.tensor_mul(sc, rs, pm_tile)
        EXP.append(exp_t)
        SCALE.append(sc)

    # Phase 4: out rows = expT.T @ v (all bf16). Each row tile's expT
    # transposes are emitted just before its matmuls: row 0's output stream
    # starts while rows 1-3 exp chains are still finishing on ACT/DVE.
    # Accumulators rotate over 6 PSUM banks (2 o-banks + the 4 energy banks,
    # dead after the exp pass). Epilogue is split across engines: ACT does
    # the PSUM read + pm/sum scale writing bf16, DVE adds the bf16 residual
    # at 16-bit 2x rate. Results stage into a [128, 4096] bf16 tile shipped
    # as 512/256/256 KB DMAs to shorten the kernel tail.
    EXPT = expt_pool.tile([P, MT, C], BF16, name="expt", tag="expt")

    def emit_expt_block(mi):
        tp = t_psum.tile([P, MT, P], BF16, tag="tp", name=f"tpx{mi}")
        for mj in range(MT):
            nc.tensor.transpose(tp[:, mj, :], EXP[mi][:, mj * P:(mj + 1) * P],
                                identity_bf)
        nc.vector.tensor_copy(out=EXPT[:, :, mi * P:(mi + 1) * P], in_=tp)

    emit_expt_block(0)
    for mi in range(MT):
        if mi + 1 < MT:
            emit_expt_block(mi + 1)  # one row ahead: copy overlaps mi's matmuls
        ob = out_pool.tile([P, HW], BF16, tag="ob")
        for cidx in range(NCHUNKS):
            slot = (mi * NCHUNKS + cidx) % 6
            if slot < 4:
                o_ps = e_psum.tile([P, NCH], F32, name=f"ops{slot}", tag=f"e{slot}")
            else:
                o_ps = o_psum.tile([P, NCH], F32, name=f"ops{slot}", tag="ops")
            for mj in range(MT):
                nc.tensor.matmul(
                    o_ps,
                    lhsT=EXPT[:, mj, mi * P:(mi + 1) * P],
                    rhs=V[:, mj, cidx * NCH:(cidx + 1) * NCH],
                    start=(mj == 0),
                    stop=(mj == MT - 1),
                )
            obc = ob[:, cidx * NCH:(cidx + 1) * NCH]
            nc.scalar.mul(obc, o_ps, SCALE[mi])
            nc.vector.tensor_add(
                obc, obc, V[:, mi, cidx * NCH:(cidx + 1) * NCH]
            )
            if cidx == NCHUNKS // 2 - 1:
                nc.sync.dma_start(
                    out=out[mi * P:(mi + 1) * P, :HW // 2], in_=ob[:, :HW // 2]
                )
            elif cidx == NCHUNKS - 3:
                nc.sync.dma_start(
                    out=out[mi * P:(mi + 1) * P, HW // 2:HW * 3 // 4],
                    in_=ob[:, HW // 2:HW * 3 // 4],
                )
        nc.sync.dma_start(
            out=out[mi * P:(mi + 1) * P, HW * 3 // 4:], in_=ob[:, HW * 3 // 4:]
        )


def build_nc(reps: int = 1) -> bass.Bass:
    # bacc.Bacc (not raw bass.Bass): its compile() pass legalizes multi-sem
    # waits into explicit event-semaphore instructions (walrus allows only one
    # sync wait per TPB instruction).
    nc = bacc.Bacc("TRN2", debug=False)
    x = nc.dram_tensor("x", [C, HW], F32, kind="ExternalInput").ap()
    pm = nc.dram_tensor("para_mu", [1], F32, kind="ExternalInput").ap()
    out = nc.dram_tensor("out", [C, HW], BF16, kind="ExternalOutput").ap()
    with tile.TileContext(nc) as tc, ExitStack() as ctx:
        _body(ctx, tc, out, x, pm, reps=reps)
    nc.compile()
    return nc


_nc_cache = None


def run(x: np.ndarray, para_mu: np.ndarray, **spmd_kwargs):
    """Run on 8 NeuronCores; returns (output [8,512,64,64], BassKernelResults)."""
    global _nc_cache
    x = np.ascontiguousarray(np.asarray(x, dtype=np.float32))
    pm = np.ascontiguousarray(np.asarray(para_mu, dtype=np.float32).reshape(1))
    assert x.shape == (N, C, H, W), x.shape
    if _nc_cache is None:
        _nc_cache = build_nc()
    in_maps = [
        {"x": x[n].reshape(C, HW), "para_mu": pm} for n in range(N)
    ]
    res = run_bass_kernel_spmd(_nc_cache, in_maps, core_ids=list(range(N)), **spmd_kwargs)
    out = np.stack(
        [np.asarray(res.results[n]["out"]).astype(np.float32).reshape(C, H, W)
         for n in range(N)]
    )
    return out, res


def kernel(x: np.ndarray, para_mu: np.ndarray) -> np.ndarray:
    out, _ = run(x, para_mu)
    return out


# revision 12
# speedup vs baseline: 1.1835x; 1.0660x over previous
"""Trainium2 Bass kernel for nn_CAM (channel attention module).

Reference computation (per batch element n):
    v = x[n].reshape(C, H*W)                      # [512, 4096]
    energy = v @ v.T                              # [512, 512]
    attn = softmax(energy, axis=-1)
    out = attn @ v                                # [512, 4096]
    result = para_mu * out + x[n]

Sharding: data-parallel over batch N=8, one batch element per NeuronCore
(8 cores). Everything is core-local — no collectives.

v2 kernel strategy (per core) — bf16 pipeline:
  1. Input lands as bf16 via SWDGE cast-DMA (f32 HBM read, bf16 SBUF write):
     8 column-slab DMAs into a single [128, 4, 4096] tile so the transpose
     stream starts after ~1/8 of the data. rel-err budget is 2e-2; bf16
     everywhere costs ~2e-3.
  2. vT column slabs via TensorE transposes in bf16 (1 cycle/row, vs 1.5 for
     f32r); the 4 transposes of one slab land in one PSUM tile and move to
     SBUF with one DVE copy (16-bit 2x rate), pipelined one slab ahead.
  3. Energy = vT.T @ vT, m-outer and symmetric (row tile m computes column
     blocks j >= [0,1,2,2][m]); missing lower blocks are transposed twins.
     E[0] completes right after the transpose stream so row 0's softmax
     overlaps rows 1-3's energy matmuls. Energy rows stay f32 for softmax.
  4. Row softmax: reduce_max (negated) -> Exp activation writing bf16 with
     f32 accumulated row sum -> reciprocal. exp rows stay unnormalized; the
     epilogue scale carries para_mu/rowsum.
  5. Output matmul runs fp8e4 DoubleRow (0.5 cycles/row): exp is written
     fp8 by ACT, expT transposed in fp8; the rhs is an fp8 copy of v made by
     a gpsimd SBUF->SBUF cast-DMA (keeps ACT/DVE free). The residual path
     stays bf16, so fp8's ~6% quantization only touches the para_mu-scaled
     attention term (~1e-3 of the result scale). Accumulators rotate over 6
     PSUM banks.
  6. Epilogue: ACT scales PSUM -> bf16, DVE adds the bf16 residual at 16-bit
     2x rate; the output ships to HBM as bf16 (half the store traffic; the
     host upcasts to f32).
  7. Benchmark loop is 2x-unrolled with the V/V8 tiles double-buffered, so
     rep i+1's input cast-DMA and fp8 derivation overlap rep i's output
     matmuls instead of serializing behind the last residual read of V.
"""

import sys

if "/opt/trn_rl_repo" not in sys.path:
    sys.path.insert(0, "/opt/trn_rl_repo")

from contextlib import ExitStack

import numpy as np

import concourse.bass as bass
import concourse.mybir as mybir
import concourse.tile as tile
from concourse import bacc
from concourse.bass_utils import run_bass_kernel_spmd
from concourse.masks import make_identity

N, C, H, W = 8, 512, 64, 64
HW = H * W            # 4096
P = 128               # partitions
MT = C // P           # 4 row tiles of the channel dim
KT = HW // P          # 32 contraction tiles for the energy matmul
NCH = 512             # free-dim chunk for the output matmul (one PSUM bank)
NCHUNKS = HW // NCH   # 8
F32 = mybir.dt.float32
BF16 = mybir.dt.bfloat16
F8 = mybir.dt.float8e4
DR = mybir.MatmulPerfMode.DoubleRow


def _body(ctx: ExitStack, tc: "tile.TileContext", out: bass.AP, x: bass.AP, pm: bass.AP,
          reps: int = 1):
    nc = tc.nc
    consts = ctx.enter_context(tc.tile_pool(name="consts", bufs=1))
    v_pool = ctx.enter_context(tc.tile_pool(name="v", bufs=2))
    v8_pool = ctx.enter_context(tc.tile_pool(name="v8", bufs=2))
    vt_pool = ctx.enter_context(tc.tile_pool(name="vt", bufs=1))
    exp_pool = ctx.enter_context(tc.tile_pool(name="exp", bufs=1))
    expt_pool = ctx.enter_context(tc.tile_pool(name="expt", bufs=1))
    stat_pool = ctx.enter_context(tc.tile_pool(name="stats", bufs=1))
    out_pool = ctx.enter_context(tc.tile_pool(name="ob", bufs=2))
    e_psum = ctx.enter_context(tc.tile_pool(name="e_ps", bufs=1, space="PSUM"))
    t_psum = ctx.enter_context(tc.tile_pool(name="t_ps", bufs=2, space="PSUM"))
    o_psum = ctx.enter_context(tc.tile_pool(name="o_ps", bufs=2, space="PSUM"))

    identity = consts.tile([P, P], F32)
    nc.vector.memset(identity, 0.0)
    make_identity(nc, identity, nomemset=True)
    # bf16/fp8 twins for transpose-mode matmuls of bf16/fp8 data (1 cycle/row).
    identity_bf = consts.tile([P, P], BF16)
    nc.vector.tensor_copy(out=identity_bf, in_=identity)
    identity_f8 = consts.tile([P, P], F8)
    nc.vector.tensor_copy(out=identity_f8, in_=identity)

    # emitted after make_identity: the gpsimd queue is serial, and this DMA
    # ahead of affine_select would delay the first transposes
    pm_tile = consts.tile([P, 1], F32)
    nc.gpsimd.dma_start(out=pm_tile, in_=pm.to_broadcast((P, 1)))

    pools = (consts, v_pool, v8_pool, vt_pool, exp_pool, expt_pool, stat_pool,
             out_pool, e_psum, t_psum, o_psum)
    ids = (identity, identity_bf, identity_f8)
    if reps > 1:
        # Benchmark mode: execute the body `reps` times in one NEFF via a
        # hardware loop so per-rep time is measurable over dispatch overhead.
        # 2x-unrolled so the double-buffered pools rotate: rep i+1's input
        # DMA overlaps rep i's compute.
        assert reps % 2 == 0, reps
        with tc.For_i(0, reps // 2, 1, hint_engines=(mybir.EngineType.PE,
                                                     mybir.EngineType.DVE,
                                                     mybir.EngineType.Activation)):
            _phases(tc, out, x, pm_tile, *ids, *pools)
            _phases(tc, out, x, pm_tile, *ids, *pools)
    else:
        _phases(tc, out, x, pm_tile, *ids, *pools)


def _phases(tc, out, x, pm_tile, identity, identity_bf, identity_f8,
            consts, v_pool, v8_pool, vt_pool, exp_pool, expt_pool, stat_pool,
            out_pool, e_psum, t_psum, o_psum):
    nc = tc.nc
    # Load v as bf16 in natural layout: one [128, 4, 4096] tile ([p, m, w],
    # channel row-tile m on the free axis). The SWDGE cast-DMA reads f32 from
    # HBM and writes bf16. One DMA per row tile (contiguous 16 KB/partition
    # reads -> minimal Q7 descriptor emission); in the 2x-unrolled loop these
    # prefetch under the previous rep's compute, so load ramp is off the
    # steady-state critical path. The fp8 twin for the DoubleRow output
    # matmul is derived by SBUF->SBUF cast-DMAs, also on the DMA engines.
    V = v_pool.tile([P, MT, HW], BF16, name="v", tag="v")
    V8 = v8_pool.tile([P, MT, HW], F8, name="v8", tag="v8")
    xv = x.rearrange("(m p) w -> p m w", p=P)
    for m in range(MT):
        nc.gpsimd.dma_start(out=V[:, m, :], in_=xv[:, m, :])
    for m in range(MT):
        nc.gpsimd.dma_start(out=V8[:, m, :], in_=V[:, m, :])

    # Phase 1: per k-block, transpose the [512, 128] column slab of v into
    # vts[:, k, :] [128, 512], then accumulate energy[m] += vT[k][:, m].T @
    # vT[k]. The 4 transposes of one k-block land in a single [128, 4, 128]
    # PSUM tile and move to SBUF with ONE DVE copy.
    # Energy is symmetric: row tile m only computes column blocks j >=
    # SYM_LO[m] (m=3 widened to 2 blocks to stay matmul-stream bound rather
    # than weight-load bound). Missing lower blocks are transposes of
    # computed upper blocks.
    SYM_LO = [0, 1, 2, 2]
    E = [e_psum.tile([P, C], F32, name=f"e{m}", tag=f"e{m}") for m in range(MT)]
    # vt slabs stay resident (32 KB/partition bf16) and energy runs m-outer:
    # E[0] finishes as soon as the transpose stream ends, so the whole
    # softmax/expT chain for row tile 0 hides behind the energy matmuls of
    # row tiles 1-3. Transposes are software-pipelined one k ahead of the
    # m=0 matmuls to cover the vt PSUM->SBUF copy latency.
    vts = vt_pool.tile([P, KT, C], BF16, name="vts", tag="vts")
    for k in range(KT + 1):
        if k < KT:
            tp = t_psum.tile([P, MT, P], BF16, tag="tp")
            for m in range(MT):
                nc.tensor.transpose(
                    tp[:, m, :], V[:, m, k * P:(k + 1) * P], identity_bf
                )
            nc.vector.tensor_copy(
                out=vts[:, k, :].rearrange("p (m q) -> p m q", m=MT), in_=tp
            )
        if k >= 1:
            kk = k - 1
            nc.tensor.matmul(
                E[0],
                lhsT=vts[:, kk, 0:P],
                rhs=vts[:, kk, :],
                start=(kk == 0),
                stop=(kk == KT - 1),
            )
    for m in range(1, MT):
        lo = SYM_LO[m] * P
        for k in range(KT):
            nc.tensor.matmul(
                E[m][:, lo:],
                lhsT=vts[:, k, m * P:(m + 1) * P],
                rhs=vts[:, k, lo:],
                start=(k == 0),
                stop=(k == KT - 1),
            )

    # Per row tile mi: reconstruct full energy row in SBUF (copy computed
    # part + transposed twins of missing lower blocks), row softmax stats,
    # then (phase 4) expT transposes + output matmuls + fused epilogue.
    RECON = {0: [], 1: [(1, 0)], 2: [(2, 0), (2, 1)], 3: [(3, 0), (3, 1)]}
    # PSUM->SBUF copies of every computed energy part happen up front: the
    # phase-4 accumulator rotation below reuses the energy banks, so they
    # must all be drained before the first output matmuls run. Energy rows
    # stay f32 (absolute logit errors turn into exp-scale errors).
    E_sb = []
    for m in range(MT):
        esb = exp_pool.tile([P, C], F32, name=f"esb{m}", tag=f"esb{m}")
        nc.vector.tensor_copy(out=esb[:, SYM_LO[m] * P:], in_=E[m][:, SYM_LO[m] * P:])
        E_sb.append(esb)
    # Softmax stats for every row tile, hoisted ahead of phase 4 so the
    # DVE's in-order queue never makes a later row's stats wait behind an
    # earlier row's epilogue. exp rows stay unnormalized (epilogue carries
    # pm/sum) and are written bf16 for the output matmul.
    EXP = []
    SCALE = []
    for mi in range(MT):
        for ti, tj in RECON[mi]:
            tp = t_psum.tile([P, MT, P], F32, tag="tp")
            nc.tensor.transpose(tp[:, 0, :], E_sb[tj][:, ti * P:(ti + 1) * P], identity)
            nc.vector.tensor_copy(out=E_sb[ti][:, tj * P:(tj + 1) * P], in_=tp[:, 0, :])
        neg_max = stat_pool.tile([P, 1], F32, tag=f"negm{mi}")
        nc.vector.tensor_reduce(
            out=neg_max,
            in_=E_sb[mi],
            op=mybir.AluOpType.max,
            axis=mybir.AxisListType.X,
            negate=True,
        )
        exp_t = exp_pool.tile([P, C], BF16, name=f"exp{mi}", tag=f"exp{mi}")
        s_t = stat_pool.tile([P, 1], F32, tag=f"s{mi}")
        nc.scalar.activation(
            out=exp_t,
            in_=E_sb[mi],
            func=mybir.ActivationFunctionType.Exp,
            bias=neg_max,
            scale=1.0,
            accum_out=s_t,
        )
        rs = stat_pool.tile([P, 1], F32, tag=f"rs{mi}")
        nc.vector.reciprocal(rs, s_t)
        sc = stat_pool.tile([P, 1], F32, tag=f"sc{mi}")
        nc.vector.tensor_mul(sc, rs, pm_tile)
        EXP.append(exp_t)
        SCALE.append(sc)

    # Phase 4: out rows = expT.T @ v in fp8e4 DoubleRow (contraction 256 per
    # matmul, 0.5 cycles/row). Each row tile's expT transposes are emitted
    # just before its matmuls: row 0's output stream starts while rows 1-3
    # exp chains are still finishing on ACT/DVE. Accumulators rotate over 6
    # PSUM banks (2 o-banks + the 4 energy banks, dead after the exp pass).
    # Epilogue is split across engines: ACT does the PSUM read + pm/sum
    # scale writing bf16, DVE adds the bf16 residual at 16-bit 2x rate.
    # Results stage into a [128, 4096] bf16 tile shipped as 512/256/256 KB
    # DMAs to shorten the kernel tail.
    EXPT = expt_pool.tile([P, MT, C], F8, name="expt", tag="expt")

    def emit_expt_block(mi):
        # transpose in bf16 (fp8 PSUM matmul outputs fail the BIR verifier);
        # the PSUM->SBUF DVE copy does the fp8 cast.
        tp = t_psum.tile([P, MT, P], BF16, tag="tp", name=f"tpx{mi}")
        for mj in range(MT):
            nc.tensor.transpose(tp[:, mj, :], EXP[mi][:, mj * P:(mj + 1) * P],
                                identity_bf)
        nc.vector.tensor_copy(out=EXPT[:, :, mi * P:(mi + 1) * P], in_=tp)

    emit_expt_block(0)
    for mi in range(MT):
        if mi + 1 < MT:
            emit_expt_block(mi + 1)  # one row ahead: copy overlaps mi's matmuls
        ob = out_pool.tile([P, HW], BF16, tag="ob")
        for cidx in range(NCHUNKS):
            slot = (mi * NCHUNKS + cidx) % 6
            if slot < 4:
                o_ps = e_psum.tile([P, NCH], F32, name=f"ops{slot}", tag=f"e{slot}")
            else:
                o_ps = o_psum.tile([P, NCH], F32, name=f"ops{slot}", tag="ops")
            for j in range(MT // 2):
                nc.tensor.matmul(
                    o_ps,
                    lhsT=EXPT[:, 2 * j:2 * j + 2, mi * P:(mi + 1) * P],
                    rhs=V8[:, 2 * j:2 * j + 2, cidx * NCH:(cidx + 1) * NCH],
                    start=(j == 0),
                    stop=(j == MT // 2 - 1),
                    perf_mode=DR,
                )
            obc = ob[:, cidx * NCH:(cidx + 1) * NCH]
            nc.scalar.mul(obc, o_ps, SCALE[mi])
            nc.vector.tensor_add(
                obc, obc, V[:, mi, cidx * NCH:(cidx + 1) * NCH]
            )
            if cidx == NCHUNKS // 2 - 1:
                nc.sync.dma_start(
                    out=out[mi * P:(mi + 1) * P, :HW // 2], in_=ob[:, :HW // 2]
                )
            elif cidx == NCHUNKS - 3:
                nc.sync.dma_start(
                    out=out[mi * P:(mi + 1) * P, HW // 2:HW * 3 // 4],
                    in_=ob[:, HW // 2:HW * 3 // 4],
                )
        nc.sync.dma_start(
            out=out[mi * P:(mi + 1) * P, HW * 3 // 4:], in_=ob[:, HW * 3 // 4:]
        )


def build_nc(reps: int = 1) -> bass.Bass:
    # bacc.Bacc (not raw bass.Bass): its compile() pass legalizes multi-sem
    # waits into explicit event-semaphore instructions (walrus allows only one
    # sync wait per TPB instruction).
    nc = bacc.Bacc("TRN2", debug=False)
    x = nc.dram_tensor("x", [C, HW], F32, kind="ExternalInput").ap()
    pm = nc.dram_tensor("para_mu", [1], F32, kind="ExternalInput").ap()
    out = nc.dram_tensor("out", [C, HW], BF16, kind="ExternalOutput").ap()
    with tile.TileContext(nc) as tc, ExitStack() as ctx:
        _body(ctx, tc, out, x, pm, reps=reps)
    nc.compile()
    return nc


_nc_cache = None


def run(x: np.ndarray, para_mu: np.ndarray, **spmd_kwargs):
    """Run on 8 NeuronCores; returns (output [8,512,64,64], BassKernelResults)."""
    global _nc_cache
    x = np.ascontiguousarray(np.asarray(x, dtype=np.float32))
    pm = np.ascontiguousarray(np.asarray(para_mu, dtype=np.float32).reshape(1))
    assert x.shape == (N, C, H, W), x.shape
    if _nc_cache is None:
        _nc_cache = build_nc()
    in_maps = [
        {"x": x[n].reshape(C, HW), "para_mu": pm} for n in range(N)
    ]
    res = run_bass_kernel_spmd(_nc_cache, in_maps, core_ids=list(range(N)), **spmd_kwargs)
    out = np.stack(
        [np.asarray(res.results[n]["out"]).astype(np.float32).reshape(C, H, W)
         for n in range(N)]
    )
    return out, res


def kernel(x: np.ndarray, para_mu: np.ndarray) -> np.ndarray:
    out, _ = run(x, para_mu)
    return out
